# revision 20
# baseline (speedup 1.0000x reference)
"""Multi-head attention (B=4, S=2048, D=1024, H=16, Dk=64) on 8 trn2 cores.

Sharding: data-parallel over batch (4) x sequence-split over S (2). Core
(2b+half) handles batch b, query rows half*1024:(half+1)*1024, in TRUE
(unrotated) sequence order. Each core computes K/V projections only for its
OWN half of the sequence; the halves are exchanged between the pair cores
with a pairwise AllGather (replica groups [[0,1],[2,3],[4,5],[6,7]]) through
internal DRAM, which removes the K/V projection duplication of the v1 kernel
(~131K PE stream-columns ≈ 90us/core) at the cost of a 4.2MB gather that
overlaps the projection prologue.

Per-core layout (all matmul inputs bf16, fp32 PSUM accumulation):
  xh[d, s]  = own-half x^T, [8 x (128, 1024)] (host-transposed, true order)
  K-own:  kout[m] = Wk^T xh (+bk) -> DMA -> kx_in[m];  AllGather -> kx_out
  V-own:  vout[j] = xh^T Wv (+bv, 65-col per-head layout with ones) ->
          vx_in[j]; AllGather -> vx_out
  kt[m][128, 2048] and vp[st][128, 1040] are DMA-loaded from the gathered
  kx_out/vx_out in true order (identical on both pair cores).
  QT[m] = Wq^T xh (+bq): m=0 in the prologue, m=t+1 paced as PE filler
  during attention pair t.
  Attention per head pair t (heads 2t, 2t+1), 512-col s_q chunks, per 2
  s_k-tiles g: [8 x score K=64 N=256 psum-bank-rotated][2 x exp][4 x PV
  K=128 N=512, lagged 2 tiles] -- unchanged from v1 (see git history for
  the measured rationale: same-K runs contiguous, banks interleaved).
  normalize via fp16 reciprocal + Pool-engine partition_broadcast; out
  projection (first half as pair-7 filler) -> DMA out rows 0:1024.

Measured (bench3, matched interleaved rounds, K-loop slope): v1 baseline
587us -> v2 (this file) see below; correctness vs fp32 reference 5.6e-3.
"""

import sys

sys.path.insert(0, "/opt/trn_rl_repo")

import numpy as np
import ml_dtypes

import concourse.bass as bass
import concourse.bacc as bacc
import concourse.tile as tile
import concourse.mybir as mybir
from concourse.bass_utils import run_bass_kernel_spmd

BF16 = mybir.dt.bfloat16
F32 = mybir.dt.float32
F16 = mybir.dt.float16

import os

NOEXP = bool(int(os.environ.get("MHA_NOEXP", "0")))
PACE = bool(int(os.environ.get("MHA_PACE", "1")))
LOCALKV = bool(int(os.environ.get("MHA_LOCALKV", "0")))  # ablation: skip CC, use stale kx_out
QPRO = bool(int(os.environ.get("MHA_QPRO", "1")))  # all Q proj in prologue (CC slack) vs paced filler

B, S, D, H, DK = 4, 2048, 1024, 16, 64
SH = S // 2          # own-half sequence rows per core
P = 128
NT_D = D // P        # 8 tiles along d / d'
NT_S = S // P        # 16 tiles along s
NT_SH = SH // P      # 8 own-half s tiles
NCH_Q = SH // 512    # 2 free-dim chunks for own-half s_q
VROW = H * (DK + 1)  # 1040: per-head 65 columns (64 V + 1 ones)
PAIRS = [[0, 1], [2, 3], [4, 5], [6, 7]]
LAG = 2


def build_kernel(loop_iters=1):
    nc = bacc.Bacc("TRN2", target_bir_lowering=False, debug=False, num_devices=8)

    x_d = nc.dram_tensor("x", [D, SH], BF16, kind="ExternalInput")  # own-half x^T
    wq_d = nc.dram_tensor("wq", [D, D], BF16, kind="ExternalInput")
    wk_d = nc.dram_tensor("wk", [D, D], BF16, kind="ExternalInput")
    wv_d = nc.dram_tensor("wv", [D, D], BF16, kind="ExternalInput")
    wo_d = nc.dram_tensor("wo", [D, D], BF16, kind="ExternalInput")
    bq_d = nc.dram_tensor("bq", [P, D // P], F32, kind="ExternalInput")
    bk_d = nc.dram_tensor("bk", [P, D // P], F32, kind="ExternalInput")
    bv_d = nc.dram_tensor("bv", [1, D], BF16, kind="ExternalInput")
    bo_d = nc.dram_tensor("bo", [1, D], BF16, kind="ExternalInput")
    out_d = nc.dram_tensor("out", [SH, D], F32, kind="ExternalOutput")

    # exchange staging: own-half K/V -> pairwise AllGather -> true-order full
    kx_in = nc.dram_tensor("kx_in", [NT_D, P, SH], BF16, kind="Internal")
    kx_out = nc.dram_tensor("kx_out", [2, NT_D, P, SH], BF16, kind="Internal")
    vx_in = nc.dram_tensor("vx_in", [NT_SH, P, VROW], BF16, kind="Internal")
    vx_out = nc.dram_tensor("vx_out", [2, NT_SH, P, VROW], BF16, kind="Internal")

    with tile.TileContext(nc) as tc:
        from contextlib import ExitStack

        with ExitStack() as ctx:
            st = build_load(ctx, tc, nc, x_d, wq_d, wk_d, wv_d,
                            bq_d, bk_d, bv_d, bo_d)
            st.update(kx_in=kx_in, kx_out=kx_out, vx_in=vx_in, vx_out=vx_out,
                      wo_d=wo_d, out_d=out_d)
            load_wo(nc, st)
            if loop_iters > 1:
                # benchmarking: collectives cannot live inside control flow,
                # so run projections+staging+CC once up front (fills kx_out/
                # vx_out with correct data), then loop the full body with the
                # CC instruction itself elided. Per-iteration time includes
                # everything except CC wire time (bounded separately).
                build_kv_own(tc, nc, st, do_cc=True)
                with tc.For_i(0, loop_iters, 1):
                    build_kv_own(tc, nc, st, do_cc=False)
                    build_attn(tc, nc, st)
            else:
                build_kv_own(tc, nc, st, do_cc=not LOCALKV)
                build_attn(tc, nc, st)
    nc.compile()
    return nc


def build_load(ctx, tc, nc, x_d, wq_d, wk_d, wv_d, bq_d, bk_d, bv_d, bo_d):
    const = ctx.enter_context(tc.tile_pool(name="const", bufs=1))
    qkv = ctx.enter_context(tc.tile_pool(name="qkv", bufs=1))
    wpool = ctx.enter_context(tc.tile_pool(name="w", bufs=1))
    xt_pool = ctx.enter_context(tc.tile_pool(name="xt", bufs=1))
    outp = ctx.enter_context(tc.tile_pool(name="outp", bufs=2))
    epool = ctx.enter_context(tc.tile_pool(name="epool", bufs=5))
    rpool = ctx.enter_context(tc.tile_pool(name="rpool", bufs=2))
    # PSUM: sp 2x2 banks + pv0/pv1 + ps x2 = 8 banks exactly
    spsum = ctx.enter_context(tc.tile_pool(name="spsum", bufs=2, space="PSUM"))
    pvpsum = ctx.enter_context(tc.tile_pool(name="pvpsum", bufs=1, space="PSUM"))
    pspsum = ctx.enter_context(tc.tile_pool(name="pspsum", bufs=2, space="PSUM"))

    # ---- constants ----
    bqs = const.tile([P, NT_D], F32, tag="bqs")
    nc.sync.dma_start(bqs[:, :], bq_d[:, :])
    bks = const.tile([P, NT_D], F32, tag="bks")
    nc.sync.dma_start(bks[:, :], bk_d[:, :])
    bvr = const.tile([1, D], BF16, tag="bvr")
    nc.sync.dma_start(bvr[:, :], bv_d[:, :])
    bor = const.tile([1, D], BF16, tag="bor")
    nc.sync.dma_start(bor[:, :], bo_d[:, :])
    # bias rows broadcast across partitions on the Pool engine (K=1
    # ones-matmuls are ~1141 PE cycles each; this is free by comparison)
    bvb = const.tile([P, D], BF16, tag="bvb")
    nc.gpsimd.partition_broadcast(bvb[:, :], bvr[:, :])
    bob = const.tile([P, D], BF16, tag="bob")
    nc.gpsimd.partition_broadcast(bob[:, :], bor[:, :])

    # ---- weights + own-half x^T ----
    # DMA order: wk + xh first (K-own projection starts the kernel), then
    # wv (V-own), then wq (Q prologue/filler).
    wq = [wpool.tile([P, D], BF16, tag=f"wq{k}", name=f"wq{k}") for k in range(NT_D)]
    wk = [wpool.tile([P, D], BF16, tag=f"wk{k}", name=f"wk{k}") for k in range(NT_D)]
    wv = [wpool.tile([P, D], BF16, tag=f"wv{k}", name=f"wv{k}") for k in range(NT_D)]
    xt = [xt_pool.tile([P, SH], BF16, tag=f"xt{k}", name=f"xt{k}") for k in range(NT_D)]
    for k in range(NT_D):
        nc.sync.dma_start(xt[k][:, :], x_d[k * P:(k + 1) * P, :])
        nc.sync.dma_start(wk[k][:, :], wk_d[k * P:(k + 1) * P, :])
    for k in range(NT_D):
        nc.sync.dma_start(wv[k][:, :], wv_d[k * P:(k + 1) * P, :])
    for k in range(NT_D):
        nc.sync.dma_start(wq[k][:, :], wq_d[k * P:(k + 1) * P, :])

    qt = [qkv.tile([P, SH], BF16, tag=f"qt{m}", name=f"qt{m}") for m in range(NT_D)]
    kt = [qkv.tile([P, S], BF16, tag=f"kt{m}", name=f"kt{m}") for m in range(NT_D)]
    vp = [qkv.tile([P, VROW], BF16, tag=f"vp{t}", name=f"vp{t}") for t in range(NT_S)]
    ctxT = [qkv.tile([P, SH], BF16, tag=f"ctxT{m}", name=f"ctxT{m}")
            for m in range(NT_D)]

    return dict(locals())


def load_wo(nc, st):
    wpool = st["wpool"]; wo_d = st["wo_d"]
    wo = [wpool.tile([P, D], BF16, tag=f"wo{k}", name=f"wo{k}")
          for k in range(NT_D)]
    for k in range(NT_D):
        nc.sync.dma_start(wo[k][:, :], wo_d[k * P:(k + 1) * P, :])
    st["wo"] = wo


def build_kv_own(tc, nc, st, do_cc):
    """Own-half K and V projections, staged to DRAM + pairwise AllGather.

    Staging scratch: the own-half projection output is written into the kt/vp
    tiles themselves (they are fully overwritten by the gather loads later;
    Tile's WAR tracking orders stage-out DMA before gather-in DMA).
    """
    pspsum = st["pspsum"]; spsum = st["spsum"]
    bks = st["bks"]; bvb = st["bvb"]
    wk = st["wk"]; wv = st["wv"]; xt = st["xt"]
    kt = st["kt"]; vp = st["vp"]
    kx_in = st["kx_in"]; vx_in = st["vx_in"]
    kx_out = st["kx_out"]; vx_out = st["vx_out"]

    # K-own: kout[m][d', s_own] = Wk^T xh (+bk)
    for m in range(NT_D):
        ko = kt[m]
        for ch in range(NCH_Q):
            ps = pspsum.tile([P, 512], F32, tag="ps", name=f"kops{m}{ch}")
            for k in range(NT_D):
                nc.tensor.matmul(
                    ps[:, :],
                    lhsT=wk[k][:, m * P:(m + 1) * P],
                    rhs=xt[k][:, ch * 512:(ch + 1) * 512],
                    start=(k == 0), stop=(k == NT_D - 1),
                )
            nc.vector.tensor_scalar_add(
                ko[:, ch * 512:(ch + 1) * 512], ps[:, :], bks[:, m:m + 1])
        nc.sync.dma_start(kx_in[m], ko[:, 0:SH])
    if do_cc:
        nc.gpsimd.collective_compute(
            "AllGather", mybir.AluOpType.bypass, replica_groups=PAIRS,
            ins=[kx_in[:, :, :]], outs=[kx_out[:, :, :, :]],
        )

    # V-own: vout[j][s_own, (h, dk|ones)] = xh^T Wv (+bv)
    for j in range(NT_SH):
        vo = vp[j]
        nc.vector.memset(
            vo.rearrange("p (h c) -> p h c", c=DK + 1)[:, :, DK:DK + 1], 1.0)
        ps = spsum.tile([P, 1024], F32, tag="sp", name=f"vops{j}")
        for chv in range(2):
            half = ps[:, chv * 512:(chv + 1) * 512]
            for k in range(NT_D):
                nc.tensor.matmul(
                    half,
                    lhsT=xt[k][:, j * P:(j + 1) * P],
                    rhs=wv[k][:, chv * 512:(chv + 1) * 512],
                    start=(k == 0), stop=(k == NT_D - 1),
                )
        nc.vector.tensor_add(
            vo.rearrange("p (h c) -> p h c", c=DK + 1)[:, :, 0:DK],
            ps.rearrange("p (h c) -> p h c", c=DK)[:, :, :],
            bvb.rearrange("p (h c) -> p h c", c=DK)[:, :, :],
        )
        nc.sync.dma_start(vx_in[j], vo[:, :])
    if do_cc:
        nc.gpsimd.collective_compute(
            "AllGather", mybir.AluOpType.bypass, replica_groups=PAIRS,
            ins=[vx_in[:, :, :]], outs=[vx_out[:, :, :, :]],
        )


def build_attn(tc, nc, st):
    const = st["const"]; qkv = st["qkv"]; wpool = st["wpool"]
    outp = st["outp"]; epool = st["epool"]; rpool = st["rpool"]
    spsum = st["spsum"]; pvpsum = st["pvpsum"]; pspsum = st["pspsum"]
    bqs = st["bqs"]; bob = st["bob"]
    wq = st["wq"]; xt = st["xt"]
    kx_out = st["kx_out"]; vx_out = st["vx_out"]
    wo_d = st["wo_d"]; out_d = st["out_d"]

    qt = st["qt"]; kt = st["kt"]; vp = st["vp"]; ctxT = st["ctxT"]

    # gathered K/V -> SBUF (true order, identical on both pair cores),
    # first-needed tiles first: kt[0], all vp (PV consumes them within the
    # first ~20us of attention), then the remaining kt
    def load_kt(m):
        for half in range(2):
            nc.sync.dma_start(kt[m][:, half * SH:(half + 1) * SH],
                              kx_out[half, m])
    load_kt(0)
    for g in range(NT_S):
        nc.sync.dma_start(vp[g][:, :], vx_out[g // NT_SH, g % NT_SH])
    for m in range(1, NT_D):
        load_kt(m)

    wo = st["wo"]

    # ---- Q projection for one d'-tile m (prologue m=0, filler m=t+1) ----
    def q_proj_items(m):
        items = []
        for chq in range(NCH_Q):
            ps = pspsum.tile([P, 512], F32, tag="ps", name=f"qps{m}{chq}")
            for k in range(NT_D):
                items.append(lambda k=k, ps=ps, m=m, chq=chq: nc.tensor.matmul(
                    ps[:, :],
                    lhsT=wq[k][:, m * P:(m + 1) * P],
                    rhs=xt[k][:, chq * 512:(chq + 1) * 512],
                    start=(k == 0), stop=(k == NT_D - 1),
                ))
            items.append(lambda ps=ps, m=m, chq=chq: nc.vector.tensor_scalar_add(
                qt[m][:, chq * 512:(chq + 1) * 512], ps[:, :], bqs[:, m:m + 1]))
        return items

    for m in range(NT_D if QPRO else 1):
        for it in q_proj_items(m):
            it()

    # ---- output projection for one s-tile: emitted as filler closures ----
    def out_proj_items(st_range):
        items = []
        for sti in st_range:
            ot = outp.tile([P, D], F32, tag="ot", name=f"ot{sti}")
            for cho in range(2):
                po = pspsum.tile([P, 512], F32, tag="ps", name=f"po{sti}{cho}")
                for k in range(NT_D):
                    items.append(lambda k=k, po=po, sti=sti, cho=cho: nc.tensor.matmul(
                        po[:, :],
                        lhsT=ctxT[k][:, sti * P:(sti + 1) * P],
                        rhs=wo[k][:, cho * 512:(cho + 1) * 512],
                        start=(k == 0), stop=(k == NT_D - 1)))
                items.append(lambda ot=ot, po=po, cho=cho: nc.vector.tensor_add(
                    ot[:, cho * 512:(cho + 1) * 512], po[:, :],
                    bob[:, cho * 512:(cho + 1) * 512]))
                items.append(lambda sti=sti, ot=ot, cho=cho: nc.sync.dma_start(
                    out_d[sti * P:(sti + 1) * P, cho * 512:(cho + 1) * 512],
                    ot[:, cho * 512:(cho + 1) * 512]))
        return items

    # ---- attention: head pairs, pipelined scores->exp->PV with Q filler ----
    noexp_ets = None
    if NOEXP:  # timing ablation: PV reads fixed prewritten tiles, no ACT dep
        noexp_ets = [qkv.tile([P, 1024], BF16, tag=f"nxe{i}", name=f"nxe{i}")
                     for i in range(2)]
        for e in noexp_ets:
            nc.vector.memset(e[:, :], 0.5)
    filler, fi, fslots = [], 0, 0
    for t in range(NT_D):  # head pair t = heads (2t, 2t+1), share kt[t]/qt[t]
        if t + 1 < NT_D and not QPRO:
            filler = q_proj_items(t + 1)
            fi = 0
            fslots = 2 * (NT_S // 2)
        for chq in range(NCH_Q):
            if t == NT_D - 1 and chq == 1:
                # fill the last pair's second s_q chunk with the first half of
                # the output projection (needs only ctxT columns 0:512)
                while fi < len(filler):
                    filler[fi]()
                    fi += 1
                filler = out_proj_items(range(NT_D // 2))
                fi = 0
                fslots = NT_S // 2
            q0 = chq * 512
            pv0 = pvpsum.tile([P, 512], F32, tag="pv0", name=f"pv0_{t}{chq}")
            pv1 = pvpsum.tile([P, 512], F32, tag="pv1", name=f"pv1_{t}{chq}")
            ets = {}

            def emit_pv(g, pv0=pv0, pv1=pv1, ets=ets, t=t):
                for side, pvt in ((0, pv0), (1, pv1)):
                    h = 2 * t + side
                    nc.tensor.matmul(
                        pvt[0:DK + 1, :],
                        lhsT=vp[g][:, h * (DK + 1):(h + 1) * (DK + 1)],
                        rhs=ets[g][:, side * 512:(side + 1) * 512],
                        start=(g == 0), stop=(g == NT_S - 1),
                    )

            # per 2-g block: [8 x score K=64 N=256, psum writes rotating over
            # all 4 live banks][4 x PV K=128 + fillers]
            for ge in range(0, NT_S, 2):
                sps = [spsum.tile([P, 1024], F32, tag="sp",
                                  name=f"sp{t}{chq}{g}")
                       for g in (ge, ge + 1)]
                # 8 x K=64 N=256, banks A,B,C,D,A,B,C,D
                for nh in range(2):
                    for gi, g in enumerate((ge, ge + 1)):
                        for side in range(2):
                            hoff = side * DK
                            nc.tensor.matmul(
                                sps[gi][:, side * 512 + nh * 256:
                                        side * 512 + (nh + 1) * 256],
                                lhsT=kt[t][hoff:hoff + DK, g * P:(g + 1) * P],
                                rhs=qt[t][hoff:hoff + DK,
                                          q0 + nh * 256:q0 + (nh + 1) * 256],
                                start=True, stop=True,
                            )
                for gi, g in enumerate((ge, ge + 1)):
                    if NOEXP:  # timing ablation only (breaks numerics)
                        ets[g] = noexp_ets[g % 2]
                    else:
                        et = epool.tile([P, 1024], BF16, tag="et",
                                        name=f"et{t}{chq}{g}")
                        ets[g] = et
                        nc.scalar.activation(
                            et[:, :], sps[gi][:, :],
                            mybir.ActivationFunctionType.Exp, scale=0.125,
                        )
                for gg in (ge - LAG, ge - LAG + 1):
                    if gg >= 0:
                        emit_pv(gg)
                # pace fillers evenly across the pair's blocks
                if fslots > 0:
                    take = -(-(len(filler) - fi) // fslots) if PACE else 4
                    fslots -= 1
                    for _ in range(take):
                        if fi < len(filler):
                            filler[fi]()
                            fi += 1
            for gg in range(NT_S - LAG, NT_S):
                emit_pv(gg)

            # normalize both heads: 1/denom broadcast on the Pool engine
            for side, pvt in ((0, pv0), (1, pv1)):
                rr = rpool.tile([1, 512], F16, tag="rr", name=f"rr{t}{chq}{side}")
                with nc.allow_low_precision(reason="fp16 recip, 5e-4 rel"):
                    nc.vector.reciprocal(rr[:, :], pvt[DK:DK + 1, :])
                bcs = rpool.tile([DK, 512], F16, tag="bcs", name=f"bcs{t}{chq}{side}")
                nc.gpsimd.partition_broadcast(bcs[:, :], rr[:, :])
                nc.vector.tensor_mul(
                    ctxT[t][side * DK:(side + 1) * DK, q0:q0 + 512],
                    pvt[0:DK, :], bcs[:, :],
                )
        # make sure next pair's Q projection is done before its scores
        while fi < len(filler):
            filler[fi]()
            fi += 1

    # ---- output projection (second half; first half ran as pair-7 filler) ----
    for it in out_proj_items(range(NT_D // 2, NT_D)):
        it()


_NC_CACHE = None


def get_nc():
    global _NC_CACHE
    if _NC_CACHE is None:
        _NC_CACHE = build_kernel()
    return _NC_CACHE


def make_weight_map(Wq, bq, Wk, bk, Wv, bv, Wo, bo):
    """Convert the shared (all-core) weights/biases once."""
    bf = ml_dtypes.bfloat16
    return {
        "wq": np.asarray(Wq, np.float32).astype(bf),
        "wk": np.asarray(Wk, np.float32).astype(bf),
        "wv": np.asarray(Wv, np.float32).astype(bf),
        "wo": np.asarray(Wo, np.float32).astype(bf),
        "bq": np.ascontiguousarray(np.asarray(bq, np.float32).reshape(NT_D, P).T),
        "bk": np.ascontiguousarray(np.asarray(bk, np.float32).reshape(NT_D, P).T),
        "bv": np.asarray(bv, np.float32).astype(bf).reshape(1, D),
        "bo": np.asarray(bo, np.float32).astype(bf).reshape(1, D),
    }


def make_in_maps(x, Wq, bq, Wk, bk, Wv, bv, Wo, bo, **_):
    x = np.asarray(x, dtype=np.float32)
    bf = ml_dtypes.bfloat16
    wmap = make_weight_map(Wq, bq, Wk, bk, Wv, bv, Wo, bo)
    in_maps = []
    for core in range(8):
        b, half = core // 2, core % 2
        xh = x[b, half * SH:(half + 1) * SH, :]  # [SH, D], true order
        in_maps.append(
            {"x": np.ascontiguousarray(xh.T.astype(bf)), **wmap})
    return in_maps


def kernel(x, Wq, bq, Wk, bk, Wv, bv, Wo, bo, **_):
    in_maps = make_in_maps(x, Wq, bq, Wk, bk, Wv, bv, Wo, bo)

    nc = get_nc()
    res = run_bass_kernel_spmd(nc, in_maps, core_ids=list(range(8)))
    out = np.empty((B, S, D), dtype=np.float32)
    for core in range(8):
        b, half = core // 2, core % 2
        out[b, half * SH:(half + 1) * SH] = res.results[core]["out"]
    return out


if __name__ == "__main__":
    rng = np.random.default_rng(0)
    ins = {
        "x": rng.standard_normal((B, S, D), dtype=np.float32),
        "Wq": rng.standard_normal((D, D), dtype=np.float32) * 0.03,
        "bq": rng.standard_normal(D).astype(np.float32) * 0.01,
        "Wk": rng.standard_normal((D, D), dtype=np.float32) * 0.03,
        "bk": rng.standard_normal(D).astype(np.float32) * 0.01,
        "Wv": rng.standard_normal((D, D), dtype=np.float32) * 0.03,
        "bv": rng.standard_normal(D).astype(np.float32) * 0.01,
        "Wo": rng.standard_normal((D, D), dtype=np.float32) * 0.03,
        "bo": rng.standard_normal(D).astype(np.float32) * 0.01,
    }
    y = kernel(**ins)
    print(y.shape, y.dtype, float(np.abs(y).max()))


# revision 21
# speedup vs baseline: 1.0185x; 1.0185x over previous
"""Multi-head attention (B=4, S=2048, D=1024, H=16, Dk=64) on 8 trn2 cores.

Sharding: data-parallel over batch (4) x sequence-split over S (2). Core
(2b+half) handles batch b, query rows half*1024:(half+1)*1024, in TRUE
(unrotated) sequence order. Each core computes K/V projections only for its
OWN half of the sequence; the halves are exchanged between the pair cores
with a pairwise AllGather (replica groups [[0,1],[2,3],[4,5],[6,7]]) through
internal DRAM, which removes the K/V projection duplication of the v1 kernel
(~131K PE stream-columns ≈ 90us/core) at the cost of a 4.2MB gather that
overlaps the projection prologue.

Per-core layout (all matmul inputs bf16, fp32 PSUM accumulation):
  xh[d, s]  = own-half x^T, [8 x (128, 1024)] (host-transposed, true order)
  K-own:  kout[m] = Wk^T xh (+bk) -> DMA -> kx_in[m];  AllGather -> kx_out
  V-own:  vout[j] = xh^T Wv (+bv, 65-col per-head layout with ones) ->
          vx_in[j]; AllGather -> vx_out
  kt[m][128, 2048] and vp[st][128, 1040] are DMA-loaded from the gathered
  kx_out/vx_out in true order (identical on both pair cores).
  QT[m] = Wq^T xh (+bq): m=0 in the prologue, m=t+1 paced as PE filler
  during attention pair t.
  Attention per head pair t (heads 2t, 2t+1), 512-col s_q chunks, per 2
  s_k-tiles g: [8 x score K=64 N=256 psum-bank-rotated][2 x exp][4 x PV
  K=128 N=512, lagged 2 tiles] -- unchanged from v1 (see git history for
  the measured rationale: same-K runs contiguous, banks interleaved).
  normalize via fp16 reciprocal + Pool-engine partition_broadcast; out
  projection (first half as pair-7 filler) -> DMA out rows 0:1024.

Measured (bench3, matched interleaved rounds, K-loop slope): v1 baseline
587us -> v2 (this file) see below; correctness vs fp32 reference 5.6e-3.
"""

import sys

sys.path.insert(0, "/opt/trn_rl_repo")

import numpy as np
import ml_dtypes

import concourse.bass as bass
import concourse.bacc as bacc
import concourse.tile as tile
import concourse.mybir as mybir
from concourse.bass_utils import run_bass_kernel_spmd

BF16 = mybir.dt.bfloat16
F32 = mybir.dt.float32
F16 = mybir.dt.float16

import os

NOEXP = bool(int(os.environ.get("MHA_NOEXP", "0")))
PACE = bool(int(os.environ.get("MHA_PACE", "1")))
LOCALKV = bool(int(os.environ.get("MHA_LOCALKV", "0")))  # ablation: skip CC, use stale kx_out
QN = int(os.environ.get("MHA_QN", "1"))  # Q d'-tiles in prologue (rest paced as filler); more = CC slack

B, S, D, H, DK = 4, 2048, 1024, 16, 64
SH = S // 2          # own-half sequence rows per core
P = 128
NT_D = D // P        # 8 tiles along d / d'
NT_S = S // P        # 16 tiles along s
NT_SH = SH // P      # 8 own-half s tiles
NCH_Q = SH // 512    # 2 free-dim chunks for own-half s_q
VROW = H * (DK + 1)  # 1040: per-head 65 columns (64 V + 1 ones)
PAIRS = [[0, 1], [2, 3], [4, 5], [6, 7]]
LAG = 2


def build_kernel(loop_iters=1):
    nc = bacc.Bacc("TRN2", target_bir_lowering=False, debug=False, num_devices=8)

    x_d = nc.dram_tensor("x", [D, SH], BF16, kind="ExternalInput")  # own-half x^T
    wq_d = nc.dram_tensor("wq", [D, D], BF16, kind="ExternalInput")
    wk_d = nc.dram_tensor("wk", [D, D], BF16, kind="ExternalInput")
    wv_d = nc.dram_tensor("wv", [D, D], BF16, kind="ExternalInput")
    wo_d = nc.dram_tensor("wo", [D, D], BF16, kind="ExternalInput")
    bq_d = nc.dram_tensor("bq", [P, D // P], F32, kind="ExternalInput")
    bk_d = nc.dram_tensor("bk", [P, D // P], F32, kind="ExternalInput")
    bv_d = nc.dram_tensor("bv", [1, D], BF16, kind="ExternalInput")
    bo_d = nc.dram_tensor("bo", [1, D], BF16, kind="ExternalInput")
    out_d = nc.dram_tensor("out", [SH, D], F32, kind="ExternalOutput")

    # exchange staging: own-half K/V -> pairwise AllGather -> true-order full
    kx_in = nc.dram_tensor("kx_in", [NT_D, P, SH], BF16, kind="Internal")
    kx_out = nc.dram_tensor("kx_out", [2, NT_D, P, SH], BF16, kind="Internal")
    vx_in = nc.dram_tensor("vx_in", [NT_SH, P, VROW], BF16, kind="Internal")
    vx_out = nc.dram_tensor("vx_out", [2, NT_SH, P, VROW], BF16, kind="Internal")

    with tile.TileContext(nc) as tc:
        from contextlib import ExitStack

        with ExitStack() as ctx:
            st = build_load(ctx, tc, nc, x_d, wq_d, wk_d, wv_d,
                            bq_d, bk_d, bv_d, bo_d)
            st.update(kx_in=kx_in, kx_out=kx_out, vx_in=vx_in, vx_out=vx_out,
                      wo_d=wo_d, out_d=out_d)
            load_wo(nc, st)
            if loop_iters > 1:
                # benchmarking: collectives cannot live inside control flow,
                # so run projections+staging+CC once up front (fills kx_out/
                # vx_out with correct data), then loop the full body with the
                # CC instruction itself elided. Per-iteration time includes
                # everything except CC wire time (bounded separately).
                build_kv_own(tc, nc, st, do_cc=True)
                with tc.For_i(0, loop_iters, 1):
                    build_kv_own(tc, nc, st, do_cc=False)
                    build_attn(tc, nc, st)
            else:
                build_kv_own(tc, nc, st, do_cc=not LOCALKV)
                build_attn(tc, nc, st)
    nc.compile()
    return nc


def build_load(ctx, tc, nc, x_d, wq_d, wk_d, wv_d, bq_d, bk_d, bv_d, bo_d):
    const = ctx.enter_context(tc.tile_pool(name="const", bufs=1))
    qkv = ctx.enter_context(tc.tile_pool(name="qkv", bufs=1))
    wpool = ctx.enter_context(tc.tile_pool(name="w", bufs=1))
    xt_pool = ctx.enter_context(tc.tile_pool(name="xt", bufs=1))
    outp = ctx.enter_context(tc.tile_pool(name="outp", bufs=2))
    epool = ctx.enter_context(tc.tile_pool(name="epool", bufs=5))
    rpool = ctx.enter_context(tc.tile_pool(name="rpool", bufs=2))
    # PSUM: sp 2x2 banks + pv0/pv1 + ps x2 = 8 banks exactly
    spsum = ctx.enter_context(tc.tile_pool(name="spsum", bufs=2, space="PSUM"))
    pvpsum = ctx.enter_context(tc.tile_pool(name="pvpsum", bufs=1, space="PSUM"))
    pspsum = ctx.enter_context(tc.tile_pool(name="pspsum", bufs=2, space="PSUM"))

    # ---- constants ----
    bqs = const.tile([P, NT_D], F32, tag="bqs")
    nc.sync.dma_start(bqs[:, :], bq_d[:, :])
    bks = const.tile([P, NT_D], F32, tag="bks")
    nc.sync.dma_start(bks[:, :], bk_d[:, :])
    bvr = const.tile([1, D], BF16, tag="bvr")
    nc.sync.dma_start(bvr[:, :], bv_d[:, :])
    bor = const.tile([1, D], BF16, tag="bor")
    nc.sync.dma_start(bor[:, :], bo_d[:, :])
    # bias rows broadcast across partitions on the Pool engine (K=1
    # ones-matmuls are ~1141 PE cycles each; this is free by comparison)
    bvb = const.tile([P, D], BF16, tag="bvb")
    nc.gpsimd.partition_broadcast(bvb[:, :], bvr[:, :])
    bob = const.tile([P, D], BF16, tag="bob")
    nc.gpsimd.partition_broadcast(bob[:, :], bor[:, :])

    # ---- weights + own-half x^T ----
    # DMA order: wk + xh first (K-own projection starts the kernel), then
    # wv (V-own), then wq (Q prologue/filler).
    wq = [wpool.tile([P, D], BF16, tag=f"wq{k}", name=f"wq{k}") for k in range(NT_D)]
    wk = [wpool.tile([P, D], BF16, tag=f"wk{k}", name=f"wk{k}") for k in range(NT_D)]
    wv = [wpool.tile([P, D], BF16, tag=f"wv{k}", name=f"wv{k}") for k in range(NT_D)]
    xt = [xt_pool.tile([P, SH], BF16, tag=f"xt{k}", name=f"xt{k}") for k in range(NT_D)]
    for k in range(NT_D):
        nc.sync.dma_start(xt[k][:, :], x_d[k * P:(k + 1) * P, :])
        nc.sync.dma_start(wk[k][:, :], wk_d[k * P:(k + 1) * P, :])
    for k in range(NT_D):
        nc.sync.dma_start(wv[k][:, :], wv_d[k * P:(k + 1) * P, :])
    for k in range(NT_D):
        nc.sync.dma_start(wq[k][:, :], wq_d[k * P:(k + 1) * P, :])

    qt = [qkv.tile([P, SH], BF16, tag=f"qt{m}", name=f"qt{m}") for m in range(NT_D)]
    kt = [qkv.tile([P, S], BF16, tag=f"kt{m}", name=f"kt{m}") for m in range(NT_D)]
    vp = [qkv.tile([P, VROW], BF16, tag=f"vp{t}", name=f"vp{t}") for t in range(NT_S)]
    ctxT = [qkv.tile([P, SH], BF16, tag=f"ctxT{m}", name=f"ctxT{m}")
            for m in range(NT_D)]

    return dict(locals())


def load_wo(nc, st):
    wpool = st["wpool"]; wo_d = st["wo_d"]
    wo = [wpool.tile([P, D], BF16, tag=f"wo{k}", name=f"wo{k}")
          for k in range(NT_D)]
    for k in range(NT_D):
        nc.sync.dma_start(wo[k][:, :], wo_d[k * P:(k + 1) * P, :])
    st["wo"] = wo


def build_kv_own(tc, nc, st, do_cc):
    """Own-half K and V projections, staged to DRAM + pairwise AllGather.

    Staging scratch: the own-half projection output is written into the kt/vp
    tiles themselves (they are fully overwritten by the gather loads later;
    Tile's WAR tracking orders stage-out DMA before gather-in DMA).
    """
    pspsum = st["pspsum"]; spsum = st["spsum"]
    bks = st["bks"]; bvb = st["bvb"]
    wk = st["wk"]; wv = st["wv"]; xt = st["xt"]
    kt = st["kt"]; vp = st["vp"]
    kx_in = st["kx_in"]; vx_in = st["vx_in"]
    kx_out = st["kx_out"]; vx_out = st["vx_out"]

    # K-own: kout[m][d', s_own] = Wk^T xh (+bk)
    for m in range(NT_D):
        ko = kt[m]
        for ch in range(NCH_Q):
            ps = pspsum.tile([P, 512], F32, tag="ps", name=f"kops{m}{ch}")
            for k in range(NT_D):
                nc.tensor.matmul(
                    ps[:, :],
                    lhsT=wk[k][:, m * P:(m + 1) * P],
                    rhs=xt[k][:, ch * 512:(ch + 1) * 512],
                    start=(k == 0), stop=(k == NT_D - 1),
                )
            nc.vector.tensor_scalar_add(
                ko[:, ch * 512:(ch + 1) * 512], ps[:, :], bks[:, m:m + 1])
        nc.sync.dma_start(kx_in[m], ko[:, 0:SH])
    if do_cc:
        nc.gpsimd.collective_compute(
            "AllGather", mybir.AluOpType.bypass, replica_groups=PAIRS,
            ins=[kx_in[:, :, :]], outs=[kx_out[:, :, :, :]],
        )

    # V-own: vout[j][s_own, (h, dk|ones)] = xh^T Wv (+bv)
    for j in range(NT_SH):
        vo = vp[j]
        nc.vector.memset(
            vo.rearrange("p (h c) -> p h c", c=DK + 1)[:, :, DK:DK + 1], 1.0)
        ps = spsum.tile([P, 1024], F32, tag="sp", name=f"vops{j}")
        for chv in range(2):
            half = ps[:, chv * 512:(chv + 1) * 512]
            for k in range(NT_D):
                nc.tensor.matmul(
                    half,
                    lhsT=xt[k][:, j * P:(j + 1) * P],
                    rhs=wv[k][:, chv * 512:(chv + 1) * 512],
                    start=(k == 0), stop=(k == NT_D - 1),
                )
        nc.vector.tensor_add(
            vo.rearrange("p (h c) -> p h c", c=DK + 1)[:, :, 0:DK],
            ps.rearrange("p (h c) -> p h c", c=DK)[:, :, :],
            bvb.rearrange("p (h c) -> p h c", c=DK)[:, :, :],
        )
        nc.sync.dma_start(vx_in[j], vo[:, :])
    if do_cc:
        nc.gpsimd.collective_compute(
            "AllGather", mybir.AluOpType.bypass, replica_groups=PAIRS,
            ins=[vx_in[:, :, :]], outs=[vx_out[:, :, :, :]],
        )


def build_attn(tc, nc, st):
    const = st["const"]; qkv = st["qkv"]; wpool = st["wpool"]
    outp = st["outp"]; epool = st["epool"]; rpool = st["rpool"]
    spsum = st["spsum"]; pvpsum = st["pvpsum"]; pspsum = st["pspsum"]
    bqs = st["bqs"]; bob = st["bob"]
    wq = st["wq"]; xt = st["xt"]
    kx_out = st["kx_out"]; vx_out = st["vx_out"]
    wo_d = st["wo_d"]; out_d = st["out_d"]

    qt = st["qt"]; kt = st["kt"]; vp = st["vp"]; ctxT = st["ctxT"]

    # gathered K/V -> SBUF (true order, identical on both pair cores),
    # first-needed tiles first: kt[0], all vp (PV consumes them within the
    # first ~20us of attention), then the remaining kt
    def load_kt(m):
        for half in range(2):
            nc.sync.dma_start(kt[m][:, half * SH:(half + 1) * SH],
                              kx_out[half, m])
    load_kt(0)
    for g in range(NT_S):
        nc.sync.dma_start(vp[g][:, :], vx_out[g // NT_SH, g % NT_SH])
    for m in range(1, NT_D):
        load_kt(m)

    wo = st["wo"]

    # ---- Q projection for one d'-tile m (prologue m=0, filler m=t+1) ----
    def q_proj_items(m):
        items = []
        for chq in range(NCH_Q):
            ps = pspsum.tile([P, 512], F32, tag="ps", name=f"qps{m}{chq}")
            for k in range(NT_D):
                items.append(lambda k=k, ps=ps, m=m, chq=chq: nc.tensor.matmul(
                    ps[:, :],
                    lhsT=wq[k][:, m * P:(m + 1) * P],
                    rhs=xt[k][:, chq * 512:(chq + 1) * 512],
                    start=(k == 0), stop=(k == NT_D - 1),
                ))
            items.append(lambda ps=ps, m=m, chq=chq: nc.vector.tensor_scalar_add(
                qt[m][:, chq * 512:(chq + 1) * 512], ps[:, :], bqs[:, m:m + 1]))
        return items

    for m in range(QN):
        for it in q_proj_items(m):
            it()

    # ---- output projection for one s-tile: emitted as filler closures ----
    def out_proj_items(st_range):
        items = []
        for sti in st_range:
            ot = outp.tile([P, D], F32, tag="ot", name=f"ot{sti}")
            for cho in range(2):
                po = pspsum.tile([P, 512], F32, tag="ps", name=f"po{sti}{cho}")
                for k in range(NT_D):
                    items.append(lambda k=k, po=po, sti=sti, cho=cho: nc.tensor.matmul(
                        po[:, :],
                        lhsT=ctxT[k][:, sti * P:(sti + 1) * P],
                        rhs=wo[k][:, cho * 512:(cho + 1) * 512],
                        start=(k == 0), stop=(k == NT_D - 1)))
                items.append(lambda ot=ot, po=po, cho=cho: nc.vector.tensor_add(
                    ot[:, cho * 512:(cho + 1) * 512], po[:, :],
                    bob[:, cho * 512:(cho + 1) * 512]))
                items.append(lambda sti=sti, ot=ot, cho=cho: nc.sync.dma_start(
                    out_d[sti * P:(sti + 1) * P, cho * 512:(cho + 1) * 512],
                    ot[:, cho * 512:(cho + 1) * 512]))
        return items

    # ---- attention: head pairs, pipelined scores->exp->PV with Q filler ----
    noexp_ets = None
    if NOEXP:  # timing ablation: PV reads fixed prewritten tiles, no ACT dep
        noexp_ets = [qkv.tile([P, 1024], BF16, tag=f"nxe{i}", name=f"nxe{i}")
                     for i in range(2)]
        for e in noexp_ets:
            nc.vector.memset(e[:, :], 0.5)
    filler, fi, fslots = [], 0, 0
    for t in range(NT_D):  # head pair t = heads (2t, 2t+1), share kt[t]/qt[t]
        if QN <= t + 1 < NT_D:
            filler = q_proj_items(t + 1)
            fi = 0
            fslots = 2 * (NT_S // 2)
        for chq in range(NCH_Q):
            if t == NT_D - 1 and chq == 1:
                # fill the last pair's second s_q chunk with the first half of
                # the output projection (needs only ctxT columns 0:512)
                while fi < len(filler):
                    filler[fi]()
                    fi += 1
                filler = out_proj_items(range(NT_D // 2))
                fi = 0
                fslots = NT_S // 2
            q0 = chq * 512
            pv0 = pvpsum.tile([P, 512], F32, tag="pv0", name=f"pv0_{t}{chq}")
            pv1 = pvpsum.tile([P, 512], F32, tag="pv1", name=f"pv1_{t}{chq}")
            ets = {}

            def emit_pv(g, pv0=pv0, pv1=pv1, ets=ets, t=t):
                for side, pvt in ((0, pv0), (1, pv1)):
                    h = 2 * t + side
                    nc.tensor.matmul(
                        pvt[0:DK + 1, :],
                        lhsT=vp[g][:, h * (DK + 1):(h + 1) * (DK + 1)],
                        rhs=ets[g][:, side * 512:(side + 1) * 512],
                        start=(g == 0), stop=(g == NT_S - 1),
                    )

            # per 2-g block: [8 x score K=64 N=256, psum writes rotating over
            # all 4 live banks][4 x PV K=128 + fillers]
            for ge in range(0, NT_S, 2):
                sps = [spsum.tile([P, 1024], F32, tag="sp",
                                  name=f"sp{t}{chq}{g}")
                       for g in (ge, ge + 1)]
                # 8 x K=64 N=256, banks A,B,C,D,A,B,C,D
                for nh in range(2):
                    for gi, g in enumerate((ge, ge + 1)):
                        for side in range(2):
                            hoff = side * DK
                            nc.tensor.matmul(
                                sps[gi][:, side * 512 + nh * 256:
                                        side * 512 + (nh + 1) * 256],
                                lhsT=kt[t][hoff:hoff + DK, g * P:(g + 1) * P],
                                rhs=qt[t][hoff:hoff + DK,
                                          q0 + nh * 256:q0 + (nh + 1) * 256],
                                start=True, stop=True,
                            )
                for gi, g in enumerate((ge, ge + 1)):
                    if NOEXP:  # timing ablation only (breaks numerics)
                        ets[g] = noexp_ets[g % 2]
                    else:
                        et = epool.tile([P, 1024], BF16, tag="et",
                                        name=f"et{t}{chq}{g}")
                        ets[g] = et
                        nc.scalar.activation(
                            et[:, :], sps[gi][:, :],
                            mybir.ActivationFunctionType.Exp, scale=0.125,
                        )
                for gg in (ge - LAG, ge - LAG + 1):
                    if gg >= 0:
                        emit_pv(gg)
                # pace fillers evenly across the pair's blocks
                if fslots > 0:
                    take = -(-(len(filler) - fi) // fslots) if PACE else 4
                    fslots -= 1
                    for _ in range(take):
                        if fi < len(filler):
                            filler[fi]()
                            fi += 1
            for gg in range(NT_S - LAG, NT_S):
                emit_pv(gg)

            # normalize both heads: 1/denom broadcast on the Pool engine
            for side, pvt in ((0, pv0), (1, pv1)):
                rr = rpool.tile([1, 512], F16, tag="rr", name=f"rr{t}{chq}{side}")
                with nc.allow_low_precision(reason="fp16 recip, 5e-4 rel"):
                    nc.vector.reciprocal(rr[:, :], pvt[DK:DK + 1, :])
                bcs = rpool.tile([DK, 512], F16, tag="bcs", name=f"bcs{t}{chq}{side}")
                nc.gpsimd.partition_broadcast(bcs[:, :], rr[:, :])
                nc.vector.tensor_mul(
                    ctxT[t][side * DK:(side + 1) * DK, q0:q0 + 512],
                    pvt[0:DK, :], bcs[:, :],
                )
        # make sure next pair's Q projection is done before its scores
        while fi < len(filler):
            filler[fi]()
            fi += 1

    # ---- output projection (second half; first half ran as pair-7 filler) ----
    for it in out_proj_items(range(NT_D // 2, NT_D)):
        it()


_NC_CACHE = None


def get_nc():
    global _NC_CACHE
    if _NC_CACHE is None:
        _NC_CACHE = build_kernel()
    return _NC_CACHE


def make_weight_map(Wq, bq, Wk, bk, Wv, bv, Wo, bo):
    """Convert the shared (all-core) weights/biases once."""
    bf = ml_dtypes.bfloat16
    return {
        "wq": np.asarray(Wq, np.float32).astype(bf),
        "wk": np.asarray(Wk, np.float32).astype(bf),
        "wv": np.asarray(Wv, np.float32).astype(bf),
        "wo": np.asarray(Wo, np.float32).astype(bf),
        "bq": np.ascontiguousarray(np.asarray(bq, np.float32).reshape(NT_D, P).T),
        "bk": np.ascontiguousarray(np.asarray(bk, np.float32).reshape(NT_D, P).T),
        "bv": np.asarray(bv, np.float32).astype(bf).reshape(1, D),
        "bo": np.asarray(bo, np.float32).astype(bf).reshape(1, D),
    }


def make_in_maps(x, Wq, bq, Wk, bk, Wv, bv, Wo, bo, **_):
    x = np.asarray(x, dtype=np.float32)
    bf = ml_dtypes.bfloat16
    wmap = make_weight_map(Wq, bq, Wk, bk, Wv, bv, Wo, bo)
    in_maps = []
    for core in range(8):
        b, half = core // 2, core % 2
        xh = x[b, half * SH:(half + 1) * SH, :]  # [SH, D], true order
        in_maps.append(
            {"x": np.ascontiguousarray(xh.T.astype(bf)), **wmap})
    return in_maps


def kernel(x, Wq, bq, Wk, bk, Wv, bv, Wo, bo, **_):
    in_maps = make_in_maps(x, Wq, bq, Wk, bk, Wv, bv, Wo, bo)

    nc = get_nc()
    res = run_bass_kernel_spmd(nc, in_maps, core_ids=list(range(8)))
    out = np.empty((B, S, D), dtype=np.float32)
    for core in range(8):
        b, half = core // 2, core % 2
        out[b, half * SH:(half + 1) * SH] = res.results[core]["out"]
    return out


if __name__ == "__main__":
    rng = np.random.default_rng(0)
    ins = {
        "x": rng.standard_normal((B, S, D), dtype=np.float32),
        "Wq": rng.standard_normal((D, D), dtype=np.float32) * 0.03,
        "bq": rng.standard_normal(D).astype(np.float32) * 0.01,
        "Wk": rng.standard_normal((D, D), dtype=np.float32) * 0.03,
        "bk": rng.standard_normal(D).astype(np.float32) * 0.01,
        "Wv": rng.standard_normal((D, D), dtype=np.float32) * 0.03,
        "bv": rng.standard_normal(D).astype(np.float32) * 0.01,
        "Wo": rng.standard_normal((D, D), dtype=np.float32) * 0.03,
        "bo": rng.standard_normal(D).astype(np.float32) * 0.01,
    }
    y = kernel(**ins)
    print(y.shape, y.dtype, float(np.abs(y).max()))


# revision 22
# speedup vs baseline: 1.0482x; 1.0292x over previous
"""Multi-head attention (B=4, S=2048, D=1024, H=16, Dk=64) on 8 trn2 cores.

Sharding: data-parallel over batch (4) x sequence-split over S (2). Core
(2b+half) handles batch b, query rows half*1024:(half+1)*1024, in TRUE
(unrotated) sequence order. Each core computes K/V projections only for its
OWN half of the sequence; the halves are exchanged between the pair cores
with a pairwise AllGather (replica groups [[0,1],[2,3],[4,5],[6,7]]) through
internal DRAM, which removes the K/V projection duplication of the v1 kernel
(~131K PE stream-columns ≈ 90us/core) at the cost of a 4.2MB gather that
overlaps the projection prologue.

Per-core layout (all matmul inputs bf16, fp32 PSUM accumulation):
  xh[d, s]  = own-half x^T, [8 x (128, 1024)] (host-transposed, true order)
  K-own:  kout[m] = Wk^T xh (+bk) -> DMA -> kx_in[m];  AllGather -> kx_out
  V-own:  vout[j] = xh^T Wv (+bv, 65-col per-head layout with ones) ->
          vx_in[j]; AllGather -> vx_out
  kt[m][128, 2048] and vp[st][128, 1040] are DMA-loaded from the gathered
  kx_out/vx_out in true order (identical on both pair cores).
  QT[m] = Wq^T xh (+bq): m=0 in the prologue, m=t+1 paced as PE filler
  during attention pair t.
  Attention per head pair t (heads 2t, 2t+1), 512-col s_q chunks, per 2
  s_k-tiles g: [8 x score K=64 N=256 psum-bank-rotated][2 x exp][4 x PV
  K=128 N=512, lagged 2 tiles] -- unchanged from v1 (see git history for
  the measured rationale: same-K runs contiguous, banks interleaved).
  normalize via fp16 reciprocal + Pool-engine partition_broadcast; out
  projection (first half as pair-7 filler) -> DMA out rows 0:1024.

Measured (bench3, matched interleaved rounds, K-loop slope): v1 baseline
587us -> v2 (this file) see below; correctness vs fp32 reference 5.6e-3.
"""

import sys

sys.path.insert(0, "/opt/trn_rl_repo")

import numpy as np
import ml_dtypes

import concourse.bass as bass
import concourse.bacc as bacc
import concourse.tile as tile
import concourse.mybir as mybir
from concourse.bass_utils import run_bass_kernel_spmd

BF16 = mybir.dt.bfloat16
F32 = mybir.dt.float32
F16 = mybir.dt.float16

import os

NOEXP = bool(int(os.environ.get("MHA_NOEXP", "0")))
PACE = bool(int(os.environ.get("MHA_PACE", "1")))
LOCALKV = bool(int(os.environ.get("MHA_LOCALKV", "0")))  # ablation: skip CC, use stale kx_out
QN = int(os.environ.get("MHA_QN", "3"))  # Q d'-tiles in prologue (rest paced as filler); more = CC slack

B, S, D, H, DK = 4, 2048, 1024, 16, 64
SH = S // 2          # own-half sequence rows per core
P = 128
NT_D = D // P        # 8 tiles along d / d'
NT_S = S // P        # 16 tiles along s
NT_SH = SH // P      # 8 own-half s tiles
NCH_Q = SH // 512    # 2 free-dim chunks for own-half s_q
VROW = H * (DK + 1)  # 1040: per-head 65 columns (64 V + 1 ones)
PAIRS = [[0, 1], [2, 3], [4, 5], [6, 7]]
LAG = 2


def build_kernel(loop_iters=1):
    nc = bacc.Bacc("TRN2", target_bir_lowering=False, debug=False, num_devices=8)

    x_d = nc.dram_tensor("x", [D, SH], BF16, kind="ExternalInput")  # own-half x^T
    wq_d = nc.dram_tensor("wq", [D, D], BF16, kind="ExternalInput")
    wk_d = nc.dram_tensor("wk", [D, D], BF16, kind="ExternalInput")
    wv_d = nc.dram_tensor("wv", [D, D], BF16, kind="ExternalInput")
    wo_d = nc.dram_tensor("wo", [D, D], BF16, kind="ExternalInput")
    bq_d = nc.dram_tensor("bq", [P, D // P], F32, kind="ExternalInput")
    bk_d = nc.dram_tensor("bk", [P, D // P], F32, kind="ExternalInput")
    bv_d = nc.dram_tensor("bv", [1, D], BF16, kind="ExternalInput")
    bo_d = nc.dram_tensor("bo", [1, D], BF16, kind="ExternalInput")
    out_d = nc.dram_tensor("out", [SH, D], F32, kind="ExternalOutput")

    # exchange staging: own-half K/V -> pairwise AllGather -> true-order full
    kx_in = nc.dram_tensor("kx_in", [NT_D, P, SH], BF16, kind="Internal")
    kx_out = nc.dram_tensor("kx_out", [2, NT_D, P, SH], BF16, kind="Internal")
    vx_in = nc.dram_tensor("vx_in", [NT_SH, P, VROW], BF16, kind="Internal")
    vx_out = nc.dram_tensor("vx_out", [2, NT_SH, P, VROW], BF16, kind="Internal")

    with tile.TileContext(nc) as tc:
        from contextlib import ExitStack

        with ExitStack() as ctx:
            st = build_load(ctx, tc, nc, x_d, wq_d, wk_d, wv_d,
                            bq_d, bk_d, bv_d, bo_d)
            st.update(kx_in=kx_in, kx_out=kx_out, vx_in=vx_in, vx_out=vx_out,
                      wo_d=wo_d, out_d=out_d)
            load_wo(nc, st)
            if loop_iters > 1:
                # benchmarking: collectives cannot live inside control flow,
                # so run projections+staging+CC once up front (fills kx_out/
                # vx_out with correct data), then loop the full body with the
                # CC instruction itself elided. Per-iteration time includes
                # everything except CC wire time (bounded separately).
                build_kv_own(tc, nc, st, do_cc=True)
                with tc.For_i(0, loop_iters, 1):
                    build_kv_own(tc, nc, st, do_cc=False)
                    build_attn(tc, nc, st)
            else:
                build_kv_own(tc, nc, st, do_cc=not LOCALKV)
                build_attn(tc, nc, st)
    nc.compile()
    return nc


def build_load(ctx, tc, nc, x_d, wq_d, wk_d, wv_d, bq_d, bk_d, bv_d, bo_d):
    const = ctx.enter_context(tc.tile_pool(name="const", bufs=1))
    qkv = ctx.enter_context(tc.tile_pool(name="qkv", bufs=1))
    wpool = ctx.enter_context(tc.tile_pool(name="w", bufs=1))
    xt_pool = ctx.enter_context(tc.tile_pool(name="xt", bufs=1))
    outp = ctx.enter_context(tc.tile_pool(name="outp", bufs=2))
    epool = ctx.enter_context(tc.tile_pool(name="epool", bufs=5))
    rpool = ctx.enter_context(tc.tile_pool(name="rpool", bufs=2))
    # PSUM: sp 2x2 banks + pv0/pv1 + ps x2 = 8 banks exactly
    spsum = ctx.enter_context(tc.tile_pool(name="spsum", bufs=2, space="PSUM"))
    pvpsum = ctx.enter_context(tc.tile_pool(name="pvpsum", bufs=1, space="PSUM"))
    pspsum = ctx.enter_context(tc.tile_pool(name="pspsum", bufs=2, space="PSUM"))

    # ---- constants ----
    bqs = const.tile([P, NT_D], F32, tag="bqs")
    nc.sync.dma_start(bqs[:, :], bq_d[:, :])
    bks = const.tile([P, NT_D], F32, tag="bks")
    nc.sync.dma_start(bks[:, :], bk_d[:, :])
    bvr = const.tile([1, D], BF16, tag="bvr")
    nc.sync.dma_start(bvr[:, :], bv_d[:, :])
    bor = const.tile([1, D], BF16, tag="bor")
    nc.sync.dma_start(bor[:, :], bo_d[:, :])
    # bias rows broadcast across partitions on the Pool engine (K=1
    # ones-matmuls are ~1141 PE cycles each; this is free by comparison)
    bvb = const.tile([P, D], BF16, tag="bvb")
    nc.gpsimd.partition_broadcast(bvb[:, :], bvr[:, :])
    bob = const.tile([P, D], BF16, tag="bob")
    nc.gpsimd.partition_broadcast(bob[:, :], bor[:, :])

    # ---- weights + own-half x^T ----
    # DMA order: wk + xh first (K-own projection starts the kernel), then
    # wv (V-own), then wq (Q prologue/filler).
    wq = [wpool.tile([P, D], BF16, tag=f"wq{k}", name=f"wq{k}") for k in range(NT_D)]
    wk = [wpool.tile([P, D], BF16, tag=f"wk{k}", name=f"wk{k}") for k in range(NT_D)]
    wv = [wpool.tile([P, D], BF16, tag=f"wv{k}", name=f"wv{k}") for k in range(NT_D)]
    xt = [xt_pool.tile([P, SH], BF16, tag=f"xt{k}", name=f"xt{k}") for k in range(NT_D)]
    for k in range(NT_D):
        nc.sync.dma_start(xt[k][:, :], x_d[k * P:(k + 1) * P, :])
        nc.sync.dma_start(wk[k][:, :], wk_d[k * P:(k + 1) * P, :])
    for k in range(NT_D):
        nc.sync.dma_start(wv[k][:, :], wv_d[k * P:(k + 1) * P, :])
    for k in range(NT_D):
        nc.sync.dma_start(wq[k][:, :], wq_d[k * P:(k + 1) * P, :])

    qt = [qkv.tile([P, SH], BF16, tag=f"qt{m}", name=f"qt{m}") for m in range(NT_D)]
    kt = [qkv.tile([P, S], BF16, tag=f"kt{m}", name=f"kt{m}") for m in range(NT_D)]
    vp = [qkv.tile([P, VROW], BF16, tag=f"vp{t}", name=f"vp{t}") for t in range(NT_S)]
    ctxT = [qkv.tile([P, SH], BF16, tag=f"ctxT{m}", name=f"ctxT{m}")
            for m in range(NT_D)]

    return dict(locals())


def load_wo(nc, st):
    wpool = st["wpool"]; wo_d = st["wo_d"]
    wo = [wpool.tile([P, D], BF16, tag=f"wo{k}", name=f"wo{k}")
          for k in range(NT_D)]
    for k in range(NT_D):
        nc.sync.dma_start(wo[k][:, :], wo_d[k * P:(k + 1) * P, :])
    st["wo"] = wo


def build_kv_own(tc, nc, st, do_cc):
    """Own-half K and V projections, staged to DRAM + pairwise AllGather.

    Staging scratch: the own-half projection output is written into the kt/vp
    tiles themselves (they are fully overwritten by the gather loads later;
    Tile's WAR tracking orders stage-out DMA before gather-in DMA).
    """
    pspsum = st["pspsum"]; spsum = st["spsum"]
    bks = st["bks"]; bvb = st["bvb"]
    wk = st["wk"]; wv = st["wv"]; xt = st["xt"]
    kt = st["kt"]; vp = st["vp"]
    kx_in = st["kx_in"]; vx_in = st["vx_in"]
    kx_out = st["kx_out"]; vx_out = st["vx_out"]

    # K-own: kout[m][d', s_own] = Wk^T xh (+bk)
    for m in range(NT_D):
        ko = kt[m]
        for ch in range(NCH_Q):
            ps = pspsum.tile([P, 512], F32, tag="ps", name=f"kops{m}{ch}")
            for k in range(NT_D):
                nc.tensor.matmul(
                    ps[:, :],
                    lhsT=wk[k][:, m * P:(m + 1) * P],
                    rhs=xt[k][:, ch * 512:(ch + 1) * 512],
                    start=(k == 0), stop=(k == NT_D - 1),
                )
            nc.vector.tensor_scalar_add(
                ko[:, ch * 512:(ch + 1) * 512], ps[:, :], bks[:, m:m + 1])
        nc.sync.dma_start(kx_in[m], ko[:, 0:SH])
    if do_cc:
        nc.gpsimd.collective_compute(
            "AllGather", mybir.AluOpType.bypass, replica_groups=PAIRS,
            ins=[kx_in[:, :, :]], outs=[kx_out[:, :, :, :]],
        )

    # V-own: vout[j][s_own, (h, dk|ones)] = xh^T Wv (+bv)
    for j in range(NT_SH):
        vo = vp[j]
        nc.vector.memset(
            vo.rearrange("p (h c) -> p h c", c=DK + 1)[:, :, DK:DK + 1], 1.0)
        ps = spsum.tile([P, 1024], F32, tag="sp", name=f"vops{j}")
        for chv in range(2):
            half = ps[:, chv * 512:(chv + 1) * 512]
            for k in range(NT_D):
                nc.tensor.matmul(
                    half,
                    lhsT=xt[k][:, j * P:(j + 1) * P],
                    rhs=wv[k][:, chv * 512:(chv + 1) * 512],
                    start=(k == 0), stop=(k == NT_D - 1),
                )
        nc.vector.tensor_add(
            vo.rearrange("p (h c) -> p h c", c=DK + 1)[:, :, 0:DK],
            ps.rearrange("p (h c) -> p h c", c=DK)[:, :, :],
            bvb.rearrange("p (h c) -> p h c", c=DK)[:, :, :],
        )
        nc.sync.dma_start(vx_in[j], vo[:, :])
    if do_cc:
        nc.gpsimd.collective_compute(
            "AllGather", mybir.AluOpType.bypass, replica_groups=PAIRS,
            ins=[vx_in[:, :, :]], outs=[vx_out[:, :, :, :]],
        )


def build_attn(tc, nc, st):
    const = st["const"]; qkv = st["qkv"]; wpool = st["wpool"]
    outp = st["outp"]; epool = st["epool"]; rpool = st["rpool"]
    spsum = st["spsum"]; pvpsum = st["pvpsum"]; pspsum = st["pspsum"]
    bqs = st["bqs"]; bob = st["bob"]
    wq = st["wq"]; xt = st["xt"]
    kx_out = st["kx_out"]; vx_out = st["vx_out"]
    wo_d = st["wo_d"]; out_d = st["out_d"]

    qt = st["qt"]; kt = st["kt"]; vp = st["vp"]; ctxT = st["ctxT"]

    # gathered K/V -> SBUF (true order, identical on both pair cores),
    # first-needed tiles first: kt[0], all vp (PV consumes them within the
    # first ~20us of attention), then the remaining kt
    def load_kt(m):
        for half in range(2):
            nc.sync.dma_start(kt[m][:, half * SH:(half + 1) * SH],
                              kx_out[half, m])
    load_kt(0)
    for g in range(NT_S):
        nc.sync.dma_start(vp[g][:, :], vx_out[g // NT_SH, g % NT_SH])
    for m in range(1, NT_D):
        load_kt(m)

    wo = st["wo"]

    # ---- Q projection for one d'-tile m (prologue m=0, filler m=t+1) ----
    def q_proj_items(m):
        items = []
        for chq in range(NCH_Q):
            ps = pspsum.tile([P, 512], F32, tag="ps", name=f"qps{m}{chq}")
            for k in range(NT_D):
                items.append(lambda k=k, ps=ps, m=m, chq=chq: nc.tensor.matmul(
                    ps[:, :],
                    lhsT=wq[k][:, m * P:(m + 1) * P],
                    rhs=xt[k][:, chq * 512:(chq + 1) * 512],
                    start=(k == 0), stop=(k == NT_D - 1),
                ))
            items.append(lambda ps=ps, m=m, chq=chq: nc.vector.tensor_scalar_add(
                qt[m][:, chq * 512:(chq + 1) * 512], ps[:, :], bqs[:, m:m + 1]))
        return items

    for m in range(QN):
        for it in q_proj_items(m):
            it()

    # ---- output projection for one s-tile: emitted as filler closures ----
    def out_proj_items(st_range):
        items = []
        for sti in st_range:
            ot = outp.tile([P, D], F32, tag="ot", name=f"ot{sti}")
            for cho in range(2):
                po = pspsum.tile([P, 512], F32, tag="ps", name=f"po{sti}{cho}")
                for k in range(NT_D):
                    items.append(lambda k=k, po=po, sti=sti, cho=cho: nc.tensor.matmul(
                        po[:, :],
                        lhsT=ctxT[k][:, sti * P:(sti + 1) * P],
                        rhs=wo[k][:, cho * 512:(cho + 1) * 512],
                        start=(k == 0), stop=(k == NT_D - 1)))
                items.append(lambda ot=ot, po=po, cho=cho: nc.vector.tensor_add(
                    ot[:, cho * 512:(cho + 1) * 512], po[:, :],
                    bob[:, cho * 512:(cho + 1) * 512]))
                items.append(lambda sti=sti, ot=ot, cho=cho: nc.sync.dma_start(
                    out_d[sti * P:(sti + 1) * P, cho * 512:(cho + 1) * 512],
                    ot[:, cho * 512:(cho + 1) * 512]))
        return items

    # ---- attention: head pairs, pipelined scores->exp->PV with Q filler ----
    noexp_ets = None
    if NOEXP:  # timing ablation: PV reads fixed prewritten tiles, no ACT dep
        noexp_ets = [qkv.tile([P, 1024], BF16, tag=f"nxe{i}", name=f"nxe{i}")
                     for i in range(2)]
        for e in noexp_ets:
            nc.vector.memset(e[:, :], 0.5)
    filler, fi, fslots = [], 0, 0
    for t in range(NT_D):  # head pair t = heads (2t, 2t+1), share kt[t]/qt[t]
        if QN <= t + 1 < NT_D:
            filler = q_proj_items(t + 1)
            fi = 0
            fslots = 2 * (NT_S // 2)
        for chq in range(NCH_Q):
            if t == NT_D - 1 and chq == 1:
                # fill the last pair's second s_q chunk with the first half of
                # the output projection (needs only ctxT columns 0:512)
                while fi < len(filler):
                    filler[fi]()
                    fi += 1
                filler = out_proj_items(range(NT_D // 2))
                fi = 0
                fslots = NT_S // 2
            q0 = chq * 512
            pv0 = pvpsum.tile([P, 512], F32, tag="pv0", name=f"pv0_{t}{chq}")
            pv1 = pvpsum.tile([P, 512], F32, tag="pv1", name=f"pv1_{t}{chq}")
            ets = {}

            def emit_pv(g, pv0=pv0, pv1=pv1, ets=ets, t=t):
                for side, pvt in ((0, pv0), (1, pv1)):
                    h = 2 * t + side
                    nc.tensor.matmul(
                        pvt[0:DK + 1, :],
                        lhsT=vp[g][:, h * (DK + 1):(h + 1) * (DK + 1)],
                        rhs=ets[g][:, side * 512:(side + 1) * 512],
                        start=(g == 0), stop=(g == NT_S - 1),
                    )

            # per 2-g block: [8 x score K=64 N=256, psum writes rotating over
            # all 4 live banks][4 x PV K=128 + fillers]
            for ge in range(0, NT_S, 2):
                sps = [spsum.tile([P, 1024], F32, tag="sp",
                                  name=f"sp{t}{chq}{g}")
                       for g in (ge, ge + 1)]
                # 8 x K=64 N=256, banks A,B,C,D,A,B,C,D
                for nh in range(2):
                    for gi, g in enumerate((ge, ge + 1)):
                        for side in range(2):
                            hoff = side * DK
                            nc.tensor.matmul(
                                sps[gi][:, side * 512 + nh * 256:
                                        side * 512 + (nh + 1) * 256],
                                lhsT=kt[t][hoff:hoff + DK, g * P:(g + 1) * P],
                                rhs=qt[t][hoff:hoff + DK,
                                          q0 + nh * 256:q0 + (nh + 1) * 256],
                                start=True, stop=True,
                            )
                for gi, g in enumerate((ge, ge + 1)):
                    if NOEXP:  # timing ablation only (breaks numerics)
                        ets[g] = noexp_ets[g % 2]
                    else:
                        et = epool.tile([P, 1024], BF16, tag="et",
                                        name=f"et{t}{chq}{g}")
                        ets[g] = et
                        nc.scalar.activation(
                            et[:, :], sps[gi][:, :],
                            mybir.ActivationFunctionType.Exp, scale=0.125,
                        )
                for gg in (ge - LAG, ge - LAG + 1):
                    if gg >= 0:
                        emit_pv(gg)
                # pace fillers evenly across the pair's blocks
                if fslots > 0:
                    take = -(-(len(filler) - fi) // fslots) if PACE else 4
                    fslots -= 1
                    for _ in range(take):
                        if fi < len(filler):
                            filler[fi]()
                            fi += 1
            for gg in range(NT_S - LAG, NT_S):
                emit_pv(gg)

            # normalize both heads: 1/denom broadcast on the Pool engine
            for side, pvt in ((0, pv0), (1, pv1)):
                rr = rpool.tile([1, 512], F16, tag="rr", name=f"rr{t}{chq}{side}")
                with nc.allow_low_precision(reason="fp16 recip, 5e-4 rel"):
                    nc.vector.reciprocal(rr[:, :], pvt[DK:DK + 1, :])
                bcs = rpool.tile([DK, 512], F16, tag="bcs", name=f"bcs{t}{chq}{side}")
                nc.gpsimd.partition_broadcast(bcs[:, :], rr[:, :])
                nc.vector.tensor_mul(
                    ctxT[t][side * DK:(side + 1) * DK, q0:q0 + 512],
                    pvt[0:DK, :], bcs[:, :],
                )
        # make sure next pair's Q projection is done before its scores
        while fi < len(filler):
            filler[fi]()
            fi += 1

    # ---- output projection (second half; first half ran as pair-7 filler) ----
    for it in out_proj_items(range(NT_D // 2, NT_D)):
        it()


_NC_CACHE = None


def get_nc():
    global _NC_CACHE
    if _NC_CACHE is None:
        _NC_CACHE = build_kernel()
    return _NC_CACHE


def make_weight_map(Wq, bq, Wk, bk, Wv, bv, Wo, bo):
    """Convert the shared (all-core) weights/biases once."""
    bf = ml_dtypes.bfloat16
    return {
        "wq": np.asarray(Wq, np.float32).astype(bf),
        "wk": np.asarray(Wk, np.float32).astype(bf),
        "wv": np.asarray(Wv, np.float32).astype(bf),
        "wo": np.asarray(Wo, np.float32).astype(bf),
        "bq": np.ascontiguousarray(np.asarray(bq, np.float32).reshape(NT_D, P).T),
        "bk": np.ascontiguousarray(np.asarray(bk, np.float32).reshape(NT_D, P).T),
        "bv": np.asarray(bv, np.float32).astype(bf).reshape(1, D),
        "bo": np.asarray(bo, np.float32).astype(bf).reshape(1, D),
    }


def make_in_maps(x, Wq, bq, Wk, bk, Wv, bv, Wo, bo, **_):
    x = np.asarray(x, dtype=np.float32)
    bf = ml_dtypes.bfloat16
    wmap = make_weight_map(Wq, bq, Wk, bk, Wv, bv, Wo, bo)
    in_maps = []
    for core in range(8):
        b, half = core // 2, core % 2
        xh = x[b, half * SH:(half + 1) * SH, :]  # [SH, D], true order
        in_maps.append(
            {"x": np.ascontiguousarray(xh.T.astype(bf)), **wmap})
    return in_maps


def kernel(x, Wq, bq, Wk, bk, Wv, bv, Wo, bo, **_):
    in_maps = make_in_maps(x, Wq, bq, Wk, bk, Wv, bv, Wo, bo)

    nc = get_nc()
    res = run_bass_kernel_spmd(nc, in_maps, core_ids=list(range(8)))
    out = np.empty((B, S, D), dtype=np.float32)
    for core in range(8):
        b, half = core // 2, core % 2
        out[b, half * SH:(half + 1) * SH] = res.results[core]["out"]
    return out


if __name__ == "__main__":
    rng = np.random.default_rng(0)
    ins = {
        "x": rng.standard_normal((B, S, D), dtype=np.float32),
        "Wq": rng.standard_normal((D, D), dtype=np.float32) * 0.03,
        "bq": rng.standard_normal(D).astype(np.float32) * 0.01,
        "Wk": rng.standard_normal((D, D), dtype=np.float32) * 0.03,
        "bk": rng.standard_normal(D).astype(np.float32) * 0.01,
        "Wv": rng.standard_normal((D, D), dtype=np.float32) * 0.03,
        "bv": rng.standard_normal(D).astype(np.float32) * 0.01,
        "Wo": rng.standard_normal((D, D), dtype=np.float32) * 0.03,
        "bo": rng.standard_normal(D).astype(np.float32) * 0.01,
    }
    y = kernel(**ins)
    print(y.shape, y.dtype, float(np.abs(y).max()))
